# revision 1
# baseline (speedup 1.0000x reference)
"""Trainium2 Bass kernel for DeepGraphConvolution message passing.

Computes, for full inputs:
    hidden  = x2 @ W.T + b
    support = segment_sum(edge_vals[:,None] * hidden[cols], rows)
    y1      = relu(support) + x1
    y2      = x2
    returns (y2, y1)

Strategy (8 NeuronCores, SPMD, no collectives needed):
    support = (A @ x2) @ W.T + deg x b       where A[i,j] = sum of edge_vals
    over edges (i<-j), deg[i] = sum of edge_vals into i.

    Nodes (rows of x1/x2/support) are sharded across the 8 cores; edges are
    partitioned by destination row (standard 1D graph partitioning).  Each
    core holds a full copy of x2 in DRAM and gathers the source rows its
    edges need with GPSIMD dma_gather (one descriptor per edge, 128 rows
    per chunk).  A per-chunk one-hot selection matrix S[k, n] =
    v_k * (dest_k == n) is built on the vector engine and the tensor engine
    accumulates  accT[f, n] += G[k, f]^T S[k, n]  over each 128-destination
    block in PSUM.  The small 256x256 weight is applied per block, the
    deg x b rank-1 term is added with a K=1 matmul, then relu + x1.

    dma_gather indices are int16, so the source table is split in two
    halves (lo: nodes [0, 32768), hi: the rest) and each block's edge list
    is ordered [lo-edges | pad | hi-edges | pad] with chunk counts fixed
    across cores (max over cores, data comes padded with (idx=0, v=0)).
"""

import os
import numpy as np

P = 128
D = 256
M = 8  # NeuronCores

_NC_CACHE: dict = {}


def _build_nc(CA, CB, C_total, S16, lo, nhi, npad, gdt_name):
    """Build the Bass program. CA/CB: per-block chunk counts (lo/hi half)."""
    import concourse.bacc as bacc
    import concourse.tile as tile
    from concourse import mybir

    NBLK = len(CA)
    gdt = getattr(mybir.dt, gdt_name)
    f32 = mybir.dt.float32

    nc = bacc.Bacc(
        "TRN2",
        target_bir_lowering=False,
        debug=False,
        num_devices=M,
    )

    x2lo = nc.dram_tensor("x2lo", [lo, D], gdt, kind="ExternalInput").ap()
    x2hi = nc.dram_tensor("x2hi", [nhi, D], gdt, kind="ExternalInput").ap()
    idx = nc.dram_tensor("idx", [P, S16], mybir.dt.int16, kind="ExternalInput").ap()
    # host-precomputed selection matrices: sdat[k, c*128+n] = v of edge slot
    # (c, k) if its local dest == n else 0
    sdat = nc.dram_tensor(
        "sdat", [P, C_total * P], gdt, kind="ExternalInput"
    ).ap()
    deg = nc.dram_tensor("deg", [1, npad], f32, kind="ExternalInput").ap()
    x1s = nc.dram_tensor("x1s", [npad, D], f32, kind="ExternalInput").ap()
    wt = nc.dram_tensor("wt", [P, 2 * D], f32, kind="ExternalInput").ap()
    bb = nc.dram_tensor("bb", [1, D], f32, kind="ExternalInput").ap()
    y1s = nc.dram_tensor("y1s", [npad, D], f32, kind="ExternalOutput").ap()

    # chunk base per block
    CAB = [a + b_ for a, b_ in zip(CA, CB)]
    chb = np.concatenate([[0], np.cumsum(CAB)]).astype(int)

    with tile.TileContext(nc) as tc:
        from contextlib import ExitStack

        with ExitStack() as ctx:
            cpool = ctx.enter_context(tc.tile_pool(name="const", bufs=1))
            gpool = ctx.enter_context(tc.tile_pool(name="gather", bufs=2))
            spool = ctx.enter_context(tc.tile_pool(name="sel", bufs=4))
            pspool = ctx.enter_context(tc.tile_pool(name="ps", bufs=2, space="PSUM"))
            apool = ctx.enter_context(tc.tile_pool(name="accs", bufs=2))
            opool = ctx.enter_context(tc.tile_pool(name="outs", bufs=3))

            # --- constants ---
            wt_sb = cpool.tile([P, 2 * D], f32)
            nc.sync.dma_start(wt_sb[:], wt[:, :])
            b_sb = cpool.tile([1, D], f32)
            nc.sync.dma_start(b_sb[:], bb[:, :])
            deg_sb = cpool.tile([1, npad], f32)
            nc.sync.dma_start(deg_sb[:], deg[:, :])
            idx_sb = cpool.tile([P, S16], mybir.dt.int16)
            nc.sync.dma_start(idx_sb[:], idx[:, :])

            for b in range(NBLK):
                nchunks = CAB[b]
                g = gpool.tile([P, nchunks, D], gdt, tag="g")
                # gather source rows for this block's edges; split calls to
                # keep descriptors per call well under the SWDGE ring size
                # >1024 descriptors in one call overruns the SWDGE ring and
                # crashes the device (firmware-fixed size; raising
                # dynamic_dma_scratch_size does NOT lift it)
                GMAX = 7
                for src_ap, cnt, coff in (
                    (x2lo, CA[b], 0),
                    (x2hi, CB[b], CA[b]),
                ):
                    for o in range(0, cnt, GMAX):
                        n = min(GMAX, cnt - o)
                        ni = n * P
                        off16 = (chb[b] + coff + o) * P // 16
                        nc.gpsimd.dma_gather(
                            out_ap=g[:, coff + o : coff + o + n, :],
                            in_ap=src_ap,
                            idxs_ap=idx_sb[:, off16 : off16 + n * 8],
                            num_idxs=ni,
                            num_idxs_reg=ni,
                            elem_size=D,
                        )

                s_blk = spool.tile([P, nchunks * P], gdt, tag="s")
                nc.sync.dma_start(
                    s_blk[:], sdat[:, chb[b] * P : (chb[b] + nchunks) * P]
                )
                pt0 = pspool.tile([P, P], f32, tag="pt0")
                pt1 = pspool.tile([P, P], f32, tag="pt1")
                for ci in range(nchunks):
                    s = s_blk[:, ci * P : (ci + 1) * P]
                    st = ci == 0
                    sp = ci == nchunks - 1
                    nc.tensor.matmul(
                        out=pt0[:], lhsT=g[:, ci, 0:P], rhs=s, start=st, stop=sp
                    )
                    nc.tensor.matmul(
                        out=pt1[:], lhsT=g[:, ci, P:D], rhs=s, start=st, stop=sp
                    )

                a0 = apool.tile([P, P], f32, tag="a0")
                a1 = apool.tile([P, P], f32, tag="a1")
                nc.vector.tensor_copy(a0[:], pt0[:])
                nc.vector.tensor_copy(a1[:], pt1[:])

                p2 = pspool.tile([P, D], f32, tag="p2")
                nc.tensor.matmul(
                    out=p2[:], lhsT=a0[:], rhs=wt_sb[:, 0:D], start=True, stop=False
                )
                nc.tensor.matmul(
                    out=p2[:], lhsT=a1[:], rhs=wt_sb[:, D : 2 * D], start=False,
                    stop=False,
                )
                nc.tensor.matmul(
                    out=p2[:],
                    lhsT=deg_sb[:, b * P : (b + 1) * P],
                    rhs=b_sb[:],
                    start=False,
                    stop=True,
                )

                r = opool.tile([P, D], f32, tag="r")
                nc.scalar.activation(r[:], p2[:], mybir.ActivationFunctionType.Relu)
                x1t = opool.tile([P, D], f32, tag="x1t")
                nc.sync.dma_start(x1t[:], x1s[b * P : (b + 1) * P, :])
                y = opool.tile([P, D], f32, tag="y")
                nc.vector.tensor_add(y[:], r[:], x1t[:])
                nc.sync.dma_start(y1s[b * P : (b + 1) * P, :], y[:])

    nc.compile()
    return nc


def _plan(rows, cols, edge_vals, x1, x2, W, b, n_nodes, lo, gdt_np):
    """Host-side sharding: per-core padded edge structure + input maps."""
    E = rows.shape[0]
    nper = n_nodes // M
    nblk = -(-nper // P)
    npad = nblk * P
    nhi = n_nodes - lo

    rows = rows.astype(np.int64)
    cols = cols.astype(np.int64)
    v = edge_vals.astype(np.float32)

    core = rows // nper
    ldest = rows - core * nper
    blk = ldest // P
    half = (cols >= lo).astype(np.int64)
    gid = (core * nblk + blk) * 2 + half
    ngroups = M * nblk * 2
    cnt = np.bincount(gid, minlength=ngroups).reshape(M, nblk, 2)

    # fixed per-block chunk counts = max over cores, ceil to 128
    CA = np.maximum(-(-cnt[:, :, 0].max(axis=0) // P), 0).astype(int)
    CB = np.maximum(-(-cnt[:, :, 1].max(axis=0) // P), 0).astype(int)
    for bi in range(nblk):
        if CA[bi] + CB[bi] == 0:
            CA[bi] = 1  # keep >=1 chunk per block so PSUM is initialized
    CAB = CA + CB
    C_total = int(CAB.sum())
    S = C_total * P  # slots per core
    S16 = S // 16

    # slot base per (block, half), identical across cores
    chb = np.concatenate([[0], np.cumsum(CAB)]).astype(np.int64)
    baseA = chb[:-1] * P
    baseB = baseA + CA * P
    base_bh = np.stack([baseA, baseB], axis=1)  # [nblk, 2]

    # rank of each edge within its (core, blk, half) group
    order = np.argsort(gid, kind="stable")
    gsort = gid[order]
    flat_cnt = cnt.reshape(-1)
    starts = np.concatenate([[0], np.cumsum(flat_cnt)])[:-1]
    rank_sorted = np.arange(E, dtype=np.int64) - starts[gsort]
    rank = np.empty(E, dtype=np.int64)
    rank[order] = rank_sorted

    pos = core * S + base_bh[blk, half] + rank  # [E] in [0, M*S)

    idx_flat = np.zeros(M * S, dtype=np.int16)
    ld_flat = np.zeros(M * S, dtype=np.float32)
    v_flat = np.zeros(M * S, dtype=np.float32)
    idx_flat[pos] = np.where(half == 1, cols - lo, cols).astype(np.int16)
    ld_flat[pos] = (ldest % P).astype(np.float32)
    v_flat[pos] = v

    # weight: wt[p, t*256+n] = W[n, t*128+p]
    wt_host = np.ascontiguousarray(
        W.astype(np.float32).T.reshape(2, P, D).transpose(1, 0, 2).reshape(P, 2 * D)
    )
    b_host = np.ascontiguousarray(b.astype(np.float32).reshape(1, D))
    x2lo_host = np.ascontiguousarray(x2[:lo].astype(gdt_np))
    x2hi_host = np.ascontiguousarray(x2[lo:].astype(gdt_np))

    slot_k = np.arange(S, dtype=np.int64) % P
    slot_c = np.arange(S, dtype=np.int64) // P

    in_maps = []
    for r in range(M):
        sl = slice(r * S, (r + 1) * S)
        idx_w = idx_flat[sl].reshape(S16, 16).T  # [16, S16]
        idx_w = np.ascontiguousarray(np.tile(idx_w, (8, 1)))  # [128, S16]
        # selection matrices: sdat[k, c*P + ldest] = v
        sdat_h = np.zeros((P, C_total * P), dtype=gdt_np)
        ld_r = ld_flat[sl].astype(np.int64)
        sdat_h[slot_k, slot_c * P + ld_r] = v_flat[sl]
        msk = core == r
        deg_h = np.zeros((1, npad), dtype=np.float32)
        deg_h[0, : npad] = np.bincount(
            ldest[msk], weights=v[msk], minlength=npad
        )[:npad].astype(np.float32)
        x1_h = np.zeros((npad, D), dtype=np.float32)
        x1_h[:nper] = x1[r * nper : (r + 1) * nper].astype(np.float32)
        in_maps.append(
            {
                "x2lo": x2lo_host,
                "x2hi": x2hi_host,
                "idx": idx_w,
                "sdat": sdat_h,
                "deg": deg_h,
                "x1s": x1_h,
                "wt": wt_host,
                "bb": b_host,
            }
        )

    meta = dict(
        CA=tuple(int(x) for x in CA),
        CB=tuple(int(x) for x in CB),
        C_total=C_total,
        S16=S16,
        lo=lo,
        nhi=nhi,
        npad=npad,
        nper=nper,
    )
    return in_maps, meta


def _get_nc(meta, gdt_name):
    key = (meta["CA"], meta["CB"], meta["S16"], meta["lo"], meta["npad"], gdt_name)
    if key not in _NC_CACHE:
        _NC_CACHE[key] = _build_nc(
            list(meta["CA"]),
            list(meta["CB"]),
            meta["C_total"],
            meta["S16"],
            meta["lo"],
            meta["nhi"],
            meta["npad"],
            gdt_name,
        )
    return _NC_CACHE[key]


def kernel(x1, x2, rows, cols, edge_vals, W, b):
    from concourse.bass_utils import run_bass_kernel_spmd

    x1 = np.asarray(x1)
    x2 = np.asarray(x2)
    rows = np.asarray(rows)
    cols = np.asarray(cols)
    edge_vals = np.asarray(edge_vals)
    W = np.asarray(W)
    b = np.asarray(b)

    n_nodes = x1.shape[0]
    gdt_name = os.environ.get("GNN_GDT", "float32")
    if gdt_name == "bfloat16":
        import ml_dtypes

        gdt_np = ml_dtypes.bfloat16
    else:
        gdt_np = np.float32

    # Both halves must fit int16 gather indices. Asymmetric split: the hi
    # half (~35% of edges, ~6 chunks/block) then fits one <=896-descriptor
    # dma_gather call per block, minimizing the ~2.5us/call fixed cost.
    lo = (n_nodes + 1) // 2 if n_nodes <= 32704 else 32704
    assert lo <= 32767 and n_nodes - lo <= 32767
    in_maps, meta = _plan(
        rows, cols, edge_vals, x1, x2, W, b, n_nodes, lo, gdt_np
    )
    nc = _get_nc(meta, gdt_name)

    res = run_bass_kernel_spmd(nc, in_maps, core_ids=list(range(M)))

    nper = meta["nper"]
    y1 = np.concatenate([res.results[r]["y1s"][:nper] for r in range(M)], axis=0)
    y2 = x2.astype(np.float32)
    return (y2, y1.astype(np.float32))



# revision 6
# speedup vs baseline: 1.0002x; 1.0002x over previous
"""Trainium2 Bass kernel for DeepGraphConvolution message passing.

Computes, for full inputs:
    hidden  = x2 @ W.T + b
    support = segment_sum(edge_vals[:,None] * hidden[cols], rows)
    y1      = relu(support) + x1
    y2      = x2
    returns (y2, y1)

Strategy (8 NeuronCores, SPMD, no collectives needed):
    support = (A @ x2) @ W.T + deg x b       where A[i,j] = sum of edge_vals
    over edges (i<-j), deg[i] = sum of edge_vals into i.

    Nodes (rows of x1/x2/support) are sharded across the 8 cores; edges are
    partitioned by destination row (standard 1D graph partitioning).  Each
    core holds a full copy of x2 in DRAM and gathers the source rows its
    edges need with GPSIMD dma_gather (one descriptor per slot, 128 rows
    per chunk).  A per-chunk selection matrix S[k, n] = sum of v over edges
    (slot k -> local dest n) is shipped from the host and the tensor engine
    accumulates  accT[f, n] += G[k, f]^T S[k, n]  over each 128-destination
    block in PSUM.  The small 256x256 weight is applied per block, the
    deg x b rank-1 term is added with a K=1 matmul, then relu + x1.

    dma_gather indices are int16, so the source table is split in two
    halves (lo: nodes [0, 32704), hi: the rest) and each block's slot list
    is ordered [lo-slots | pad | hi-slots | pad] with chunk counts fixed
    across cores (max over cores).

    GPSIMD descriptor generation (~8ns/descriptor) is the bottleneck, so:
      - slots are deduplicated per (block, half, source): one gathered row
        fans out to all its dests in the block through its S row;
      - padding slots get idx = -1: the dma_gather ucode trims trailing
        negative indices BEFORE descriptor generation, so per-core padding
        costs nothing.  Skipped slots keep stale gather-buffer contents, so
        the two persistent gather buffers are memset once at startup (a
        NaN there would poison S=0 matmul columns: NaN*0=NaN).
      - gather data + S are bf16 (halves DMA + speeds the sel-matmuls);
        PSUM accumulation and the weight application stay f32.
"""

import os
import numpy as np

P = 128
D = 256
M = 8  # NeuronCores

_NC_CACHE: dict = {}


def _build_nc(CA, CB, C_total, S16, lo, nhi, npad, gdt_name):
    """Build the Bass program. CA/CB: per-block chunk counts (lo/hi half)."""
    import concourse.bacc as bacc
    import concourse.tile as tile
    from concourse import mybir

    NBLK = len(CA)
    gdt = getattr(mybir.dt, gdt_name)
    f32 = mybir.dt.float32

    nc = bacc.Bacc(
        "TRN2",
        target_bir_lowering=False,
        debug=False,
        num_devices=M,
    )

    x2lo = nc.dram_tensor("x2lo", [lo, D], gdt, kind="ExternalInput").ap()
    x2hi = nc.dram_tensor("x2hi", [nhi, D], gdt, kind="ExternalInput").ap()
    idx = nc.dram_tensor("idx", [P, S16], mybir.dt.int16, kind="ExternalInput").ap()
    # host-precomputed selection matrices: sdat[k, c*128+n] = sum of v over
    # edges in slot (c, k) with local dest == n
    sdat = nc.dram_tensor(
        "sdat", [P, C_total * P], gdt, kind="ExternalInput"
    ).ap()
    deg = nc.dram_tensor("deg", [1, npad], f32, kind="ExternalInput").ap()
    x1s = nc.dram_tensor("x1s", [npad, D], f32, kind="ExternalInput").ap()
    wt = nc.dram_tensor("wt", [P, 2 * D], f32, kind="ExternalInput").ap()
    bb = nc.dram_tensor("bb", [1, D], f32, kind="ExternalInput").ap()
    y1s = nc.dram_tensor("y1s", [npad, D], f32, kind="ExternalOutput").ap()

    # chunk base per block
    CAB = [a + b_ for a, b_ in zip(CA, CB)]
    chb = np.concatenate([[0], np.cumsum(CAB)]).astype(int)
    CMAX = int(max(CAB))

    with tile.TileContext(nc) as tc:
        from contextlib import ExitStack

        with ExitStack() as ctx:
            cpool = ctx.enter_context(tc.tile_pool(name="const", bufs=1))
            gpool = ctx.enter_context(tc.tile_pool(name="gather", bufs=1))
            spool = ctx.enter_context(tc.tile_pool(name="sel", bufs=4))
            pspool = ctx.enter_context(tc.tile_pool(name="ps", bufs=2, space="PSUM"))
            apool = ctx.enter_context(tc.tile_pool(name="accs", bufs=2))
            opool = ctx.enter_context(tc.tile_pool(name="outs", bufs=3))

            # --- constants ---
            wt_sb = cpool.tile([P, 2 * D], f32)
            nc.sync.dma_start(wt_sb[:], wt[:, :])
            b_sb = cpool.tile([1, D], f32)
            nc.sync.dma_start(b_sb[:], bb[:, :])
            deg_sb = cpool.tile([1, npad], f32)
            nc.sync.dma_start(deg_sb[:], deg[:, :])
            idx_sb = cpool.tile([P, S16], mybir.dt.int16)
            nc.sync.dma_start(idx_sb[:], idx[:, :])

            # persistent double-buffered gather tiles; memset once so that
            # slots skipped by the -1 trailing-idx trim hold finite data
            g_a = cpool.tile([P, CMAX, D], gdt, tag="g_a")
            g_b = cpool.tile([P, CMAX, D], gdt, tag="g_b")
            gbufs = [g_a, g_b]
            for g in gbufs:
                nc.vector.memset(g[:], 0.0)

            for b in range(NBLK):
                nchunks = CAB[b]
                g = gbufs[b % 2][:, 0:nchunks, :]
                # gather source rows for this block's slots; split calls to
                # keep descriptors per call well under the SWDGE ring size
                # >1024 descriptors in one call overruns the SWDGE ring and
                # crashes the device (firmware-fixed size; raising
                # dynamic_dma_scratch_size does NOT lift it)
                GMAX = 7
                for src_ap, cnt, coff in (
                    (x2lo, CA[b], 0),
                    (x2hi, CB[b], CA[b]),
                ):
                    for o in range(0, cnt, GMAX):
                        n = min(GMAX, cnt - o)
                        ni = n * P
                        off16 = (chb[b] + coff + o) * P // 16
                        nc.gpsimd.dma_gather(
                            out_ap=g[:, coff + o : coff + o + n, :],
                            in_ap=src_ap,
                            idxs_ap=idx_sb[:, off16 : off16 + n * 8],
                            num_idxs=ni,
                            num_idxs_reg=ni,
                            elem_size=D,
                        )

                s_blk = spool.tile([P, nchunks * P], gdt, tag="s")
                nc.sync.dma_start(
                    s_blk[:], sdat[:, chb[b] * P : (chb[b] + nchunks) * P]
                )
                pt0 = pspool.tile([P, P], f32, tag="pt0")
                pt1 = pspool.tile([P, P], f32, tag="pt1")
                for ci in range(nchunks):
                    s = s_blk[:, ci * P : (ci + 1) * P]
                    st = ci == 0
                    sp = ci == nchunks - 1
                    nc.tensor.matmul(
                        out=pt0[:], lhsT=g[:, ci, 0:P], rhs=s, start=st, stop=sp
                    )
                    nc.tensor.matmul(
                        out=pt1[:], lhsT=g[:, ci, P:D], rhs=s, start=st, stop=sp
                    )

                a0 = apool.tile([P, P], f32, tag="a0")
                a1 = apool.tile([P, P], f32, tag="a1")
                nc.vector.tensor_copy(a0[:], pt0[:])
                nc.vector.tensor_copy(a1[:], pt1[:])

                p2 = pspool.tile([P, D], f32, tag="p2")
                nc.tensor.matmul(
                    out=p2[:], lhsT=a0[:], rhs=wt_sb[:, 0:D], start=True, stop=False
                )
                nc.tensor.matmul(
                    out=p2[:], lhsT=a1[:], rhs=wt_sb[:, D : 2 * D], start=False,
                    stop=False,
                )
                nc.tensor.matmul(
                    out=p2[:],
                    lhsT=deg_sb[:, b * P : (b + 1) * P],
                    rhs=b_sb[:],
                    start=False,
                    stop=True,
                )

                r = opool.tile([P, D], f32, tag="r")
                nc.scalar.activation(r[:], p2[:], mybir.ActivationFunctionType.Relu)
                x1t = opool.tile([P, D], f32, tag="x1t")
                nc.sync.dma_start(x1t[:], x1s[b * P : (b + 1) * P, :])
                y = opool.tile([P, D], f32, tag="y")
                nc.vector.tensor_add(y[:], r[:], x1t[:])
                nc.sync.dma_start(y1s[b * P : (b + 1) * P, :], y[:])

    nc.compile()
    return nc


def _plan(rows, cols, edge_vals, x1, x2, W, b, n_nodes, lo, gdt_np):
    """Host-side sharding: per-core padded slot structure + input maps.

    Slots are deduplicated per (core, block, half, source): one gathered
    row serves every edge with that source landing in the block; its S row
    carries each edge's v at that edge's local-dest column.
    """
    E = rows.shape[0]
    nper = n_nodes // M
    nblk = -(-nper // P)
    npad = nblk * P
    nhi = n_nodes - lo

    rows = rows.astype(np.int64)
    cols = cols.astype(np.int64)
    v = edge_vals.astype(np.float32)

    core = rows // nper
    ldest = rows - core * nper
    blk = ldest // P
    half = (cols >= lo).astype(np.int64)
    gid = (core * nblk + blk) * 2 + half
    ngroups = M * nblk * 2

    # dedup within each (core, blk, half) by source: slot id per edge
    if os.environ.get("GNN_DEDUP", "1") == "1":
        ekey = gid * n_nodes + cols  # unique (group, source) key
    else:
        ekey = (gid * n_nodes + cols) * E + np.arange(E, dtype=np.int64)
    ukey, slot_of_edge = np.unique(ekey, return_inverse=True)
    if os.environ.get("GNN_DEDUP", "1") == "1":
        ugid = ukey // n_nodes
        ucol = ukey % n_nodes
    else:
        ugid = ukey // E // n_nodes
        ucol = (ukey // E) % n_nodes
    uhalf = ugid & 1
    cnt = np.bincount(ugid, minlength=ngroups).reshape(M, nblk, 2)

    # fixed per-block chunk counts = max over cores, ceil to 128
    CA = np.maximum(-(-cnt[:, :, 0].max(axis=0) // P), 0).astype(int)
    CB = np.maximum(-(-cnt[:, :, 1].max(axis=0) // P), 0).astype(int)
    for bi in range(nblk):
        if CA[bi] + CB[bi] == 0:
            CA[bi] = 1  # keep >=1 chunk per block so PSUM is initialized
    CAB = CA + CB
    C_total = int(CAB.sum())
    S = C_total * P  # slots per core
    S16 = S // 16

    # slot base per (block, half), identical across cores
    chb = np.concatenate([[0], np.cumsum(CAB)]).astype(np.int64)
    baseA = chb[:-1] * P
    baseB = baseA + CA * P
    base_bh = np.stack([baseA, baseB], axis=1)  # [nblk, 2]

    # rank of each unique slot within its (core, blk, half) group
    # (ukey is sorted, so ranks are consecutive within each group)
    U = ukey.shape[0]
    gstart = np.concatenate([[0], np.cumsum(np.bincount(ugid, minlength=ngroups))])
    urank = np.arange(U, dtype=np.int64) - gstart[ugid]

    ucore = ugid // (2 * nblk)
    ublk = (ugid // 2) % nblk
    upos = ucore * S + base_bh[ublk, uhalf] + urank  # [U] in [0, M*S)

    # idx: -1 everywhere (trailing-pad slots are trimmed by the ucode
    # before descriptor generation), real slots get their source index
    pad_idx = -1 if os.environ.get("GNN_TRIM", "1") == "1" else 0
    idx_flat = np.full(M * S, pad_idx, dtype=np.int16)
    idx_flat[upos] = np.where(uhalf == 1, ucol - lo, ucol).astype(np.int16)

    # per-edge S scatter targets
    epos = upos[slot_of_edge]  # slot position of each edge
    eslot = epos % S  # slot within core
    ek = eslot % P
    ec = eslot // P

    # weight: wt[p, t*256+n] = W[n, t*128+p]
    wt_host = np.ascontiguousarray(
        W.astype(np.float32).T.reshape(2, P, D).transpose(1, 0, 2).reshape(P, 2 * D)
    )
    b_host = np.ascontiguousarray(b.astype(np.float32).reshape(1, D))
    x2lo_host = np.ascontiguousarray(x2[:lo].astype(gdt_np))
    x2hi_host = np.ascontiguousarray(x2[lo:].astype(gdt_np))

    in_maps = []
    for r in range(M):
        sl = slice(r * S, (r + 1) * S)
        idx_w = idx_flat[sl].reshape(S16, 16).T  # [16, S16]
        idx_w = np.ascontiguousarray(np.tile(idx_w, (8, 1)))  # [128, S16]
        # selection matrices: sdat[k, c*P + ldest] += v
        sdat_h = np.zeros((P, C_total * P), dtype=np.float32)
        msk = core == r
        np.add.at(
            sdat_h,
            (ek[msk], ec[msk] * P + (ldest[msk] % P)),
            v[msk],
        )
        deg_h = np.zeros((1, npad), dtype=np.float32)
        deg_h[0, : npad] = np.bincount(
            ldest[msk], weights=v[msk], minlength=npad
        )[:npad].astype(np.float32)
        x1_h = np.zeros((npad, D), dtype=np.float32)
        x1_h[:nper] = x1[r * nper : (r + 1) * nper].astype(np.float32)
        in_maps.append(
            {
                "x2lo": x2lo_host,
                "x2hi": x2hi_host,
                "idx": idx_w,
                "sdat": sdat_h.astype(gdt_np),
                "deg": deg_h,
                "x1s": x1_h,
                "wt": wt_host,
                "bb": b_host,
            }
        )

    meta = dict(
        CA=tuple(int(x) for x in CA),
        CB=tuple(int(x) for x in CB),
        C_total=C_total,
        S16=S16,
        lo=lo,
        nhi=nhi,
        npad=npad,
        nper=nper,
    )
    return in_maps, meta


def _get_nc(meta, gdt_name):
    key = (meta["CA"], meta["CB"], meta["S16"], meta["lo"], meta["npad"], gdt_name)
    if key not in _NC_CACHE:
        _NC_CACHE[key] = _build_nc(
            list(meta["CA"]),
            list(meta["CB"]),
            meta["C_total"],
            meta["S16"],
            meta["lo"],
            meta["nhi"],
            meta["npad"],
            gdt_name,
        )
    return _NC_CACHE[key]


def _gdt(gdt_name):
    if gdt_name == "bfloat16":
        import ml_dtypes

        return ml_dtypes.bfloat16
    return np.float32


def kernel(x1, x2, rows, cols, edge_vals, W, b):
    from concourse.bass_utils import run_bass_kernel_spmd

    x1 = np.asarray(x1)
    x2 = np.asarray(x2)
    rows = np.asarray(rows)
    cols = np.asarray(cols)
    edge_vals = np.asarray(edge_vals)
    W = np.asarray(W)
    b = np.asarray(b)

    n_nodes = x1.shape[0]
    gdt_name = os.environ.get("GNN_GDT", "bfloat16")
    gdt_np = _gdt(gdt_name)

    # Both halves must fit int16 gather indices. Asymmetric split: the hi
    # half (~35% of edges, ~6 chunks/block) then fits one <=896-descriptor
    # dma_gather call per block, minimizing the ~2.5us/call fixed cost.
    lo = (n_nodes + 1) // 2 if n_nodes <= 32704 else 32704
    assert lo <= 32767 and n_nodes - lo <= 32767
    in_maps, meta = _plan(
        rows, cols, edge_vals, x1, x2, W, b, n_nodes, lo, gdt_np
    )
    nc = _get_nc(meta, gdt_name)

    res = run_bass_kernel_spmd(nc, in_maps, core_ids=list(range(M)))

    nper = meta["nper"]
    y1 = np.concatenate([res.results[r]["y1s"][:nper] for r in range(M)], axis=0)
    y2 = x2.astype(np.float32)
    return (y2, y1.astype(np.float32))


# revision 9
# speedup vs baseline: 2.2340x; 2.2335x over previous
"""Trainium2 Bass kernel for DeepGraphConvolution message passing.

Computes, for full inputs:
    hidden  = x2 @ W.T + b
    support = segment_sum(edge_vals[:,None] * hidden[cols], rows)
    y1      = relu(support) + x1
    y2      = x2
    returns (y2, y1)

Strategy (8 NeuronCores, SPMD, no collectives needed):
    support = (A @ x2) @ W.T + deg x b       where A[i,j] = sum of edge_vals
    over edges (i<-j), deg[i] = sum of edge_vals into i.

    Nodes (rows of x1/x2/support) are sharded across the 8 cores; edges are
    partitioned by destination row (standard 1D graph partitioning).  Each
    core holds a full copy of x2 in DRAM and gathers the source rows its
    edges need with GPSIMD dma_gather (one descriptor per slot, 128 rows
    per chunk).  A per-chunk selection matrix S[k, n] = sum of v over edges
    (slot k -> local dest n) is shipped from the host and the tensor engine
    accumulates  accT[f, n] += G[k, f]^T S[k, n]  over each 128-destination
    block in PSUM.  The small 256x256 weight is applied per block, the
    deg x b rank-1 term is added with a K=1 matmul, then relu + x1.

    dma_gather indices are int16, so the source table is split in two
    halves (lo: nodes [0, 32704), hi: the rest) and each block's slot list
    is ordered [lo-slots | pad | hi-slots | pad] with chunk counts fixed
    across cores (max over cores).

    GPSIMD descriptor generation (~8ns/descriptor) is the bottleneck, so:
      - slots are deduplicated per (block, half, source): one gathered row
        fans out to all its dests in the block through its S row;
      - padding slots get idx = -1: the dma_gather ucode trims trailing
        negative indices BEFORE descriptor generation, so per-core padding
        costs nothing.  Skipped slots keep stale gather-buffer contents, so
        the two persistent gather buffers are memset once at startup (a
        NaN there would poison S=0 matmul columns: NaN*0=NaN).
      - gather data + S are bf16 (halves DMA + speeds the sel-matmuls);
        PSUM accumulation and the weight application stay f32.
"""

import os
import numpy as np

P = 128
D = 256
M = 8  # NeuronCores

_NC_CACHE: dict = {}


def _build_nc(CA, CB, C_total, S16, lo, nhi, npad, gdt_name):
    """Build the Bass program. CA/CB: per-block chunk counts (lo/hi half)."""
    import concourse.bacc as bacc
    import concourse.tile as tile
    from concourse import mybir

    NBLK = len(CA)
    gdt = getattr(mybir.dt, gdt_name)
    f32 = mybir.dt.float32

    nc = bacc.Bacc(
        "TRN2",
        target_bir_lowering=False,
        debug=False,
        num_devices=M,
        num_swdge_queues=4,
    )

    x2lo = nc.dram_tensor("x2lo", [lo, D], gdt, kind="ExternalInput").ap()
    x2hi = nc.dram_tensor("x2hi", [nhi, D], gdt, kind="ExternalInput").ap()
    idx = nc.dram_tensor("idx", [P, S16], mybir.dt.int16, kind="ExternalInput").ap()
    # host-precomputed selection matrices: sdat[k, c*128+n] = sum of v over
    # edges in slot (c, k) with local dest == n
    sdat = nc.dram_tensor(
        "sdat", [P, C_total * P], gdt, kind="ExternalInput"
    ).ap()
    deg = nc.dram_tensor("deg", [1, npad], f32, kind="ExternalInput").ap()
    x1s = nc.dram_tensor("x1s", [npad, D], f32, kind="ExternalInput").ap()
    wt = nc.dram_tensor("wt", [P, 2 * D], f32, kind="ExternalInput").ap()
    bb = nc.dram_tensor("bb", [1, D], f32, kind="ExternalInput").ap()
    y1s = nc.dram_tensor("y1s", [npad, D], f32, kind="ExternalOutput").ap()

    # chunk base per block
    CAB = [a + b_ for a, b_ in zip(CA, CB)]
    chb = np.concatenate([[0], np.cumsum(CAB)]).astype(int)
    CMAX = int(max(CAB))

    with tile.TileContext(nc) as tc:
        from contextlib import ExitStack

        with ExitStack() as ctx:
            cpool = ctx.enter_context(tc.tile_pool(name="const", bufs=1))
            gpool = ctx.enter_context(tc.tile_pool(name="gather", bufs=1))
            spool = ctx.enter_context(tc.tile_pool(name="sel", bufs=4))
            pspool = ctx.enter_context(tc.tile_pool(name="ps", bufs=2, space="PSUM"))
            apool = ctx.enter_context(tc.tile_pool(name="accs", bufs=2))
            opool = ctx.enter_context(tc.tile_pool(name="outs", bufs=3))

            # --- constants ---
            wt_sb = cpool.tile([P, 2 * D], f32)
            nc.sync.dma_start(wt_sb[:], wt[:, :])
            b_sb = cpool.tile([1, D], f32)
            nc.sync.dma_start(b_sb[:], bb[:, :])
            deg_sb = cpool.tile([1, npad], f32)
            nc.sync.dma_start(deg_sb[:], deg[:, :])
            idx_sb = cpool.tile([P, S16], mybir.dt.int16)
            nc.sync.dma_start(idx_sb[:], idx[:, :])

            # persistent double-buffered gather tiles; memset once so that
            # slots skipped by the -1 trailing-idx trim hold finite data
            g_a = cpool.tile([P, CMAX, D], gdt, tag="g_a")
            g_b = cpool.tile([P, CMAX, D], gdt, tag="g_b")
            gbufs = [g_a, g_b]
            for g in gbufs:
                nc.vector.memset(g[:], 0.0)

            call_no = [0]
            for b in range(NBLK):
                nchunks = CAB[b]
                g = gbufs[b % 2][:, 0:nchunks, :]
                # gather source rows for this block's slots; split calls to
                # keep descriptors per call well under the SWDGE ring size
                # >1024 descriptors in one call overruns the SWDGE ring and
                # crashes the device (firmware-fixed size; raising
                # dynamic_dma_scratch_size does NOT lift it)
                # round-robin the calls over the 4 SWDGE queues: queue_num
                # selects which GPSIMD Q7 core pair generates descriptors,
                # and the pairs run concurrently (~2.4x measured speedup)
                GMAX = 7
                for src_ap, cnt, coff in (
                    (x2lo, CA[b], 0),
                    (x2hi, CB[b], CA[b]),
                ):
                    for o in range(0, cnt, GMAX):
                        n = min(GMAX, cnt - o)
                        ni = n * P
                        off16 = (chb[b] + coff + o) * P // 16
                        nc.gpsimd.dma_gather(
                            out_ap=g[:, coff + o : coff + o + n, :],
                            in_ap=src_ap,
                            idxs_ap=idx_sb[:, off16 : off16 + n * 8],
                            num_idxs=ni,
                            num_idxs_reg=ni,
                            elem_size=D,
                            queue_num=call_no[0] % 4,
                        )
                        call_no[0] += 1

                s_blk = spool.tile([P, nchunks * P], gdt, tag="s")
                nc.sync.dma_start(
                    s_blk[:], sdat[:, chb[b] * P : (chb[b] + nchunks) * P]
                )
                pt0 = pspool.tile([P, P], f32, tag="pt0")
                pt1 = pspool.tile([P, P], f32, tag="pt1")
                for ci in range(nchunks):
                    s = s_blk[:, ci * P : (ci + 1) * P]
                    st = ci == 0
                    sp = ci == nchunks - 1
                    nc.tensor.matmul(
                        out=pt0[:], lhsT=g[:, ci, 0:P], rhs=s, start=st, stop=sp
                    )
                    nc.tensor.matmul(
                        out=pt1[:], lhsT=g[:, ci, P:D], rhs=s, start=st, stop=sp
                    )

                a0 = apool.tile([P, P], f32, tag="a0")
                a1 = apool.tile([P, P], f32, tag="a1")
                nc.vector.tensor_copy(a0[:], pt0[:])
                nc.vector.tensor_copy(a1[:], pt1[:])

                p2 = pspool.tile([P, D], f32, tag="p2")
                nc.tensor.matmul(
                    out=p2[:], lhsT=a0[:], rhs=wt_sb[:, 0:D], start=True, stop=False
                )
                nc.tensor.matmul(
                    out=p2[:], lhsT=a1[:], rhs=wt_sb[:, D : 2 * D], start=False,
                    stop=False,
                )
                nc.tensor.matmul(
                    out=p2[:],
                    lhsT=deg_sb[:, b * P : (b + 1) * P],
                    rhs=b_sb[:],
                    start=False,
                    stop=True,
                )

                r = opool.tile([P, D], f32, tag="r")
                nc.scalar.activation(r[:], p2[:], mybir.ActivationFunctionType.Relu)
                x1t = opool.tile([P, D], f32, tag="x1t")
                nc.sync.dma_start(x1t[:], x1s[b * P : (b + 1) * P, :])
                y = opool.tile([P, D], f32, tag="y")
                nc.vector.tensor_add(y[:], r[:], x1t[:])
                nc.sync.dma_start(y1s[b * P : (b + 1) * P, :], y[:])

    nc.compile()
    return nc


def _plan(rows, cols, edge_vals, x1, x2, W, b, n_nodes, lo, gdt_np):
    """Host-side sharding: per-core padded slot structure + input maps.

    Slots are deduplicated per (core, block, half, source): one gathered
    row serves every edge with that source landing in the block; its S row
    carries each edge's v at that edge's local-dest column.
    """
    E = rows.shape[0]
    nper = n_nodes // M
    nblk = -(-nper // P)
    npad = nblk * P
    nhi = n_nodes - lo

    rows = rows.astype(np.int64)
    cols = cols.astype(np.int64)
    v = edge_vals.astype(np.float32)

    core = rows // nper
    ldest = rows - core * nper
    blk = ldest // P
    half = (cols >= lo).astype(np.int64)
    gid = (core * nblk + blk) * 2 + half
    ngroups = M * nblk * 2

    # dedup within each (core, blk, half) by source: slot id per edge
    if os.environ.get("GNN_DEDUP", "1") == "1":
        ekey = gid * n_nodes + cols  # unique (group, source) key
    else:
        ekey = (gid * n_nodes + cols) * E + np.arange(E, dtype=np.int64)
    ukey, slot_of_edge = np.unique(ekey, return_inverse=True)
    if os.environ.get("GNN_DEDUP", "1") == "1":
        ugid = ukey // n_nodes
        ucol = ukey % n_nodes
    else:
        ugid = ukey // E // n_nodes
        ucol = (ukey // E) % n_nodes
    uhalf = ugid & 1
    cnt = np.bincount(ugid, minlength=ngroups).reshape(M, nblk, 2)

    # fixed per-block chunk counts = max over cores, ceil to 128
    CA = np.maximum(-(-cnt[:, :, 0].max(axis=0) // P), 0).astype(int)
    CB = np.maximum(-(-cnt[:, :, 1].max(axis=0) // P), 0).astype(int)
    for bi in range(nblk):
        if CA[bi] + CB[bi] == 0:
            CA[bi] = 1  # keep >=1 chunk per block so PSUM is initialized
    CAB = CA + CB
    C_total = int(CAB.sum())
    S = C_total * P  # slots per core
    S16 = S // 16

    # slot base per (block, half), identical across cores
    chb = np.concatenate([[0], np.cumsum(CAB)]).astype(np.int64)
    baseA = chb[:-1] * P
    baseB = baseA + CA * P
    base_bh = np.stack([baseA, baseB], axis=1)  # [nblk, 2]

    # rank of each unique slot within its (core, blk, half) group
    # (ukey is sorted, so ranks are consecutive within each group)
    U = ukey.shape[0]
    gstart = np.concatenate([[0], np.cumsum(np.bincount(ugid, minlength=ngroups))])
    urank = np.arange(U, dtype=np.int64) - gstart[ugid]

    ucore = ugid // (2 * nblk)
    ublk = (ugid // 2) % nblk
    upos = ucore * S + base_bh[ublk, uhalf] + urank  # [U] in [0, M*S)

    # idx: -1 everywhere (trailing-pad slots are trimmed by the ucode
    # before descriptor generation), real slots get their source index
    pad_idx = -1 if os.environ.get("GNN_TRIM", "1") == "1" else 0
    idx_flat = np.full(M * S, pad_idx, dtype=np.int16)
    idx_flat[upos] = np.where(uhalf == 1, ucol - lo, ucol).astype(np.int16)

    # per-edge S scatter targets
    epos = upos[slot_of_edge]  # slot position of each edge
    eslot = epos % S  # slot within core
    ek = eslot % P
    ec = eslot // P

    # weight: wt[p, t*256+n] = W[n, t*128+p]
    wt_host = np.ascontiguousarray(
        W.astype(np.float32).T.reshape(2, P, D).transpose(1, 0, 2).reshape(P, 2 * D)
    )
    b_host = np.ascontiguousarray(b.astype(np.float32).reshape(1, D))
    x2lo_host = np.ascontiguousarray(x2[:lo].astype(gdt_np))
    x2hi_host = np.ascontiguousarray(x2[lo:].astype(gdt_np))

    in_maps = []
    for r in range(M):
        sl = slice(r * S, (r + 1) * S)
        idx_w = idx_flat[sl].reshape(S16, 16).T  # [16, S16]
        idx_w = np.ascontiguousarray(np.tile(idx_w, (8, 1)))  # [128, S16]
        # selection matrices: sdat[k, c*P + ldest] += v
        sdat_h = np.zeros((P, C_total * P), dtype=np.float32)
        msk = core == r
        np.add.at(
            sdat_h,
            (ek[msk], ec[msk] * P + (ldest[msk] % P)),
            v[msk],
        )
        deg_h = np.zeros((1, npad), dtype=np.float32)
        deg_h[0, : npad] = np.bincount(
            ldest[msk], weights=v[msk], minlength=npad
        )[:npad].astype(np.float32)
        x1_h = np.zeros((npad, D), dtype=np.float32)
        x1_h[:nper] = x1[r * nper : (r + 1) * nper].astype(np.float32)
        in_maps.append(
            {
                "x2lo": x2lo_host,
                "x2hi": x2hi_host,
                "idx": idx_w,
                "sdat": sdat_h.astype(gdt_np),
                "deg": deg_h,
                "x1s": x1_h,
                "wt": wt_host,
                "bb": b_host,
            }
        )

    meta = dict(
        CA=tuple(int(x) for x in CA),
        CB=tuple(int(x) for x in CB),
        C_total=C_total,
        S16=S16,
        lo=lo,
        nhi=nhi,
        npad=npad,
        nper=nper,
    )
    return in_maps, meta


def _get_nc(meta, gdt_name):
    key = (meta["CA"], meta["CB"], meta["S16"], meta["lo"], meta["npad"], gdt_name)
    if key not in _NC_CACHE:
        _NC_CACHE[key] = _build_nc(
            list(meta["CA"]),
            list(meta["CB"]),
            meta["C_total"],
            meta["S16"],
            meta["lo"],
            meta["nhi"],
            meta["npad"],
            gdt_name,
        )
    return _NC_CACHE[key]


def _gdt(gdt_name):
    if gdt_name == "bfloat16":
        import ml_dtypes

        return ml_dtypes.bfloat16
    return np.float32


def kernel(x1, x2, rows, cols, edge_vals, W, b):
    from concourse.bass_utils import run_bass_kernel_spmd

    x1 = np.asarray(x1)
    x2 = np.asarray(x2)
    rows = np.asarray(rows)
    cols = np.asarray(cols)
    edge_vals = np.asarray(edge_vals)
    W = np.asarray(W)
    b = np.asarray(b)

    n_nodes = x1.shape[0]
    gdt_name = os.environ.get("GNN_GDT", "bfloat16")
    gdt_np = _gdt(gdt_name)

    # Both halves must fit int16 gather indices. Asymmetric split: the hi
    # half (~35% of edges, ~6 chunks/block) then fits one <=896-descriptor
    # dma_gather call per block, minimizing the ~2.5us/call fixed cost.
    lo = (n_nodes + 1) // 2 if n_nodes <= 32704 else 32704
    assert lo <= 32767 and n_nodes - lo <= 32767
    in_maps, meta = _plan(
        rows, cols, edge_vals, x1, x2, W, b, n_nodes, lo, gdt_np
    )
    nc = _get_nc(meta, gdt_name)

    res = run_bass_kernel_spmd(nc, in_maps, core_ids=list(range(M)))

    nper = meta["nper"]
    y1 = np.concatenate([res.results[r]["y1s"][:nper] for r in range(M)], axis=0)
    y2 = x2.astype(np.float32)
    return (y2, y1.astype(np.float32))
